# revision 25
# baseline (speedup 1.0000x reference)
"""Single-head causal attention on 8 Trainium2 NeuronCores (Bass/Tile).

Reference: q = x@wq, k = x@wk, v = x@wv  (x: [32, 768, 256], w*: [256, 64])
           out = softmax(causal(q k^T / 8)) @ v        -> [32, 768, 64]

Sharding: data-parallel over batch, 4 samples per core, no collectives.

Per-sample dataflow (all matmul operands float32r = full-rate fp32 PE mode,
fp32 PSUM accumulation, ~2.5e-4 end-to-end max rel err):
  - x is pre-transposed on the host (xT[c, t]) so every matmul operand is
    already in the layout the PE wants (it contracts over partitions);
    the device never transposes anything.
  - qT/kT [64, 768] = w^T @ xT with the tiny weights stationary.
  - v natural [768, 64] from stationary xT blocks; two ones-columns are
    appended to v so the PV matmul also emits the softmax row sums.
  - scoresT[j, i] (keys on partitions) feeds exp-scores straight into the
    PV matmul as the stationary operand.
  - causal: only lower-triangular 128x128 blocks are computed (21 of 36);
    diagonal blocks are masked by a 0/1 multiply after exp.
  - softmax skips max-subtraction: scores here are bounded (|s| < ~3), so
    exp is numerically safe and the row sums stay O(1..768).
  - exp (ScalarE) is fused with the PSUM->SBUF move; the softmax division
    is one reciprocal + one multiply fused into the PV PSUM->SBUF move.
  - One combined 3.1 MB input DMA + one combined output DMA per step, on
    the two separate HWDGE rings (sync + scalar).
  - PV for sample b is emitted after sample b+1's projections/scores so
    the in-order PE stream never waits on exp.

Infrastructure notes: this walrus build accepts at most ONE sync-wait per
instruction, so a post-pass hoists extra waits onto same-engine NoOps.
ScalarE must run a single activation function (table reloads cost ~100 us)
and GPSIMD compute is avoided entirely for the same reason.

Set PV_BF16=1 to run the v/PV matmuls in bf16: ~1.25x faster end-to-end,
max rel err ~3.5e-3 (off by default; float32r keeps 2.5e-4).
"""
import numpy as np

import bass_rust
import concourse.bass as bass
import concourse.mybir as mybir
import concourse.tile as tile
from concourse.bass_utils import run_bass_kernel_spmd
from concourse.vector_clock import ScopedClock

F32 = mybir.dt.float32
F32R = mybir.dt.float32r
BF16 = mybir.dt.bfloat16
MMDT = F32R  # matmul operand dtype: float32r = full-rate PE, ~1.6e-4 rel err
import os
PVDT = BF16 if os.environ.get("PV_BF16") else MMDT  # v/PV matmul dtype

N_CORES = 8
B, T, C, H = 32, 768, 256, 64
BPC = B // N_CORES  # samples per core
NJ = T // 128  # 128-wide j/i chunks per sample
SCALE = 1.0 / np.sqrt(H)

# free-dim offsets of each j-chunk's row of exp-scores in the e tile
E_OFF = np.concatenate([[0], np.cumsum([T - 128 * jc for jc in range(NJ)])])
E_TOT = int(E_OFF[NJ])  # 2688


# --- workaround: this walrus build rejects instructions carrying more than
# one sync-wait command. Tile emits multi-waits freely (joins, final drain).
# Legalize post-hoc: hoist all but the last wait of each instruction onto
# same-engine NoOps inserted just before it (per-engine program order makes
# this semantically identical).
def _legalize_waits(nc):
    n_fix = 0
    for f in nc.m.functions:
        for bb in f.blocks:
            out = []
            for ins in bb.instructions:
                si = ins.sync_info
                if si is not None and si.on_wait and len(si.on_wait) > 1:
                    waits = list(si.on_wait)
                    for w in waits[:-1]:
                        nop = mybir.InstNoOp(
                            name=f"waitfix-{n_fix}", engine=ins.engine
                        )
                        nop.sync_info = bass_rust.SyncInfo(
                            on_wait=[w], on_update=[]
                        )
                        out.append(nop)
                        n_fix += 1
                    si.on_wait = [waits[-1]]
                out.append(ins)
            bb.instructions[:] = out
    return n_fix


def _spans(width, maxw=512):
    """Split [0, width) into chunks of at most maxw, avoiding tails < 256
    (float32r matmuls below 256 moving columns run at quarter rate)."""
    out = []
    o = 0
    while o < width:
        rem = width - o
        if rem > maxw and rem < maxw + 256:
            w = rem - 256
        else:
            w = min(maxw, rem)
        out.append((o, w))
        o += w
    return out


def _emit_front(nc, pools, b, xt_all, o_all, phase=3):
    """Emit sample b's q/k/v projections, scoresT and exp.  Returns the state
    the deferred PV stage needs (or None when a probe phase ends early)."""
    (x_pool, qk_pool, v_pool, e_pool, o_pool, r_pool,
     qk_psp, v_psp, s_psp, pv_psp, consts) = pools
    wq_sb, wk_sb, wv_sb, mask_sb, ones_sb, xt_d, out_d = consts

    xt_sb = xt_all[:, b]  # [128, 2, 768]
    o_sb = o_all[:, b]    # [128, 6, 64]
    if phase == 0:
        nc.vector.tensor_copy(
            o_sb[:], xt_sb[:, 0, 0 : NJ * H].rearrange("p (c h) -> p c h", h=H))
        return None

    # ---- qT [64, 768] and kT [64, 768] (walrus requires weight/fmap at the
    # same SBUF start partition, so both live at partition base 0)
    q_sb = qk_pool.tile([64, T], MMDT, tag="qsb")
    k_sb = qk_pool.tile([64, T], MMDT, tag="ksb")
    for w_sb, dst in ((wq_sb, q_sb), (wk_sb, k_sb)):
        for i0, w in _spans(T):
            ps = qk_psp.tile([64, 512], F32, tag="qkps")
            for k in range(2):
                nc.tensor.matmul(
                    ps[:, 0:w],
                    w_sb[:, k, :],
                    xt_sb[:, k, i0 : i0 + w],
                    start=(k == 0),
                    stop=(k == 1),
                )
            nc.vector.tensor_copy(dst[:, i0 : i0 + w], ps[:, 0:w])

    # ---- v natural [t, h] + ones columns: v_sb [128, 6, 66]
    if PVDT is BF16:
        xtv_sb = v_pool.tile([128, 2, T], BF16, tag="xtv")
        nc.vector.tensor_copy(xtv_sb[:], xt_sb[:])
    else:
        xtv_sb = xt_sb
    v_ps = v_psp.tile([128, NJ * H], F32, tag="vps")
    for jc in range(NJ):
        for k in range(2):
            nc.tensor.matmul(
                v_ps[:, jc * H : (jc + 1) * H],
                xtv_sb[:, k, jc * 128 : (jc + 1) * 128],
                wv_sb[:, k, :],
                start=(k == 0),
                stop=(k == 1),
            )
    v_sb = v_pool.tile([128, NJ, H + 2], PVDT, tag="vsb")
    nc.vector.tensor_copy(
        v_sb[:, :, H : H + 2],
        ones_sb[:].rearrange("p (a b) -> p a b", b=2),
    )
    nc.vector.tensor_copy(
        v_sb[:, :, 0:H],
        v_ps[:].rearrange("p (c h) -> p c h", h=H),
    )
    if phase == 1:
        nc.vector.tensor_copy(o_sb[:], v_sb[:, :, 0:H])
        return None

    # ---- scoresT[j, i] = k_j . q_i / 8, exp, causal mask on diagonal blocks
    e_sb = e_pool.tile([128, NJ, T], PVDT, tag="esb")
    for jc in range(NJ):
        ibase = 128 * jc
        kT = k_sb[:, ibase : ibase + 128]
        for i0, w in _spans(T - ibase):
            s_ps = s_psp.tile([128, 512], F32, tag="sps")
            nc.tensor.matmul(
                s_ps[:, 0:w],
                kT,
                q_sb[:, ibase + i0 : ibase + i0 + w],
                start=True,
                stop=True,
            )
            nc.scalar.activation(
                e_sb[:, jc, i0 : i0 + w],
                s_ps[:, 0:w],
                mybir.ActivationFunctionType.Exp,
                scale=float(SCALE),
            )
        nc.vector.tensor_mul(
            e_sb[:, jc, 0:128], e_sb[:, jc, 0:128], mask_sb[:]
        )
    if phase == 2:
        nc.vector.tensor_copy(
            o_sb[:], e_sb[:, 0 : NJ * H].rearrange("p (c h) -> p c h", h=H))
        return None
    return (e_sb, v_sb, o_sb)


def _emit_pv(nc, pools, state):
    """PV matmuls + softmax normalization for a sample whose front stage was
    emitted earlier (software pipelining keeps the PE stream stall-free)."""
    (x_pool, qk_pool, v_pool, e_pool, o_pool, r_pool,
     qk_psp, v_psp, s_psp, pv_psp, consts) = pools
    e_sb, v_sb, o_sb = state
    r_sb = r_pool.tile([128, NJ], F32, tag="rsb")
    # 3 ic-chains in flight across 3 PSUM banks, interleaved by jc so
    # consecutive PE matmuls hit different banks (same-bank accumulation
    # serializes fill/drain).
    for half in range(2):
        ics = range(3 * half, 3 * half + 3)
        tiles = {ic: pv_psp.tile([128, H + 2], F32, tag="pvps", name=f"pvps_{ic}") for ic in ics}
        for jc in range(NJ):
            for ic in ics:
                if jc <= ic:
                    nc.tensor.matmul(
                        tiles[ic][:],
                        e_sb[:, jc, 128 * (ic - jc) : 128 * (ic - jc) + 128],
                        v_sb[:, jc, :],
                        start=(jc == 0),
                        stop=(jc == ic),
                    )
        for ic in ics:
            nc.vector.reciprocal(r_sb[:, ic : ic + 1], tiles[ic][:, H : H + 1])
            nc.vector.tensor_scalar_mul(
                o_sb[:, ic, :], tiles[ic][:, 0:H], r_sb[:, ic : ic + 1]
            )


def build(repeats=1, phase=3):
    """Build the SPMD Bass program. repeats>1 wraps the whole per-core body
    in a hardware loop (for timing)."""
    nc = bass.Bass("TRN2", target_bir_lowering=False, debug=False, num_devices=N_CORES)

    xt_d = nc.dram_tensor("xt", [BPC, 128, 2, T], MMDT, kind="ExternalInput")
    wq_d = nc.dram_tensor("wq", [128, 2, H], MMDT, kind="ExternalInput")
    wk_d = nc.dram_tensor("wk", [128, 2, H], MMDT, kind="ExternalInput")
    wv_d = nc.dram_tensor("wv", [128, 2, H], MMDT, kind="ExternalInput")  # cast on load if PVDT != MMDT
    out_d = nc.dram_tensor("out", [BPC, T, H], F32, kind="ExternalOutput")

    mask01 = np.triu(np.ones((128, 128), dtype=np.float32))
    mask_d = nc.inline_tensor(mask01, name="mask01")
    ones_d = nc.inline_tensor(np.ones((128, NJ * 2), dtype=np.float32), name="ones")

    with tile.TileContext(nc) as tc:
        with (
            tc.tile_pool(name="const", bufs=1) as cpool,
            tc.tile_pool(name="x", bufs=2) as x_pool,
            tc.tile_pool(name="qk", bufs=2) as qk_pool,
            tc.tile_pool(name="v", bufs=2) as v_pool,
            tc.tile_pool(name="e", bufs=2) as e_pool,
            tc.tile_pool(name="o", bufs=2) as o_pool,
            tc.tile_pool(name="r", bufs=2) as r_pool,
            tc.tile_pool(name="qkps", bufs=1, space=bass.MemorySpace.PSUM) as qk_psp,
            tc.tile_pool(name="vps", bufs=1, space=bass.MemorySpace.PSUM) as v_psp,
            tc.tile_pool(name="sps", bufs=2, space=bass.MemorySpace.PSUM) as s_psp,
            tc.tile_pool(name="pvps", bufs=3, space=bass.MemorySpace.PSUM) as pv_psp,
        ):
            wq_sb = cpool.tile([128, 2, H], MMDT)
            wk_sb = cpool.tile([128, 2, H], MMDT)
            wv_sb = cpool.tile([128, 2, H], PVDT)
            mask_sb = cpool.tile([128, 128], PVDT)
            ones_sb = cpool.tile([128, NJ * 2], PVDT)
            nc.sync.dma_start(wq_sb[:], wq_d[:])
            nc.sync.dma_start(wk_sb[:], wk_d[:])
            (nc.gpsimd if PVDT is not MMDT else nc.sync).dma_start(wv_sb[:], wv_d[:])
            nc.gpsimd.dma_start(mask_sb[:], mask_d[:])
            nc.gpsimd.dma_start(ones_sb[:], ones_d[:])

            consts = (wq_sb, wk_sb, wv_sb, mask_sb, ones_sb, xt_d, out_d)
            pools = (x_pool, qk_pool, v_pool, e_pool, o_pool, r_pool,
                     qk_psp, v_psp, s_psp, pv_psp, consts)

            def body():
                xt_all = x_pool.tile([128, BPC, 2, T], MMDT, tag="xtall")
                nc.sync.dma_start(
                    xt_all[:], xt_d[:].rearrange("b p k t -> p b k t"))
                o_all = o_pool.tile([128, BPC, NJ, H], F32, tag="oall")
                pending = None
                for b in range(BPC):
                    st = _emit_front(nc, pools, b, xt_all, o_all, phase)
                    if pending is not None:
                        _emit_pv(nc, pools, pending)
                    pending = st
                if pending is not None:
                    _emit_pv(nc, pools, pending)
                nc.scalar.dma_start(
                    out_d[:].rearrange("b (a p) h -> p b a h", p=128), o_all[:])

            if repeats == 1:
                body()
            else:
                with tc.For_i(0, repeats, 1):
                    body()
    _legalize_waits(nc)
    return nc


def _prep_inputs(x, wq, wk, wv):
    x = np.asarray(x, dtype=np.float32)
    # xT per sample with c split into 2 partition chunks:
    # xt[b, p, k, t] = x[b, t, 128k + p]
    xt = np.ascontiguousarray(x.reshape(B, T, 2, 128).transpose(0, 3, 2, 1))

    def packw(w):
        w = np.asarray(w, dtype=np.float32)
        return np.ascontiguousarray(w.reshape(2, 128, H).transpose(1, 0, 2))

    return xt, packw(wq), packw(wk), packw(wv)


_NC_CACHE = {}


def _get_nc(repeats=1):
    if repeats not in _NC_CACHE:
        _NC_CACHE[repeats] = build(repeats)
    return _NC_CACHE[repeats]


def run(x, wq, wk, wv, repeats=1):
    xt, wqp, wkp, wvp = _prep_inputs(x, wq, wk, wv)
    nc = _get_nc(repeats)
    in_maps = [
        {"xt": xt[c * BPC : (c + 1) * BPC], "wq": wqp, "wk": wkp, "wv": wvp}
        for c in range(N_CORES)
    ]
    res = run_bass_kernel_spmd(nc, in_maps, core_ids=list(range(N_CORES)))
    return np.concatenate([res.results[c]["out"] for c in range(N_CORES)], axis=0)


def kernel(x, wq, wk, wv):
    return run(x, wq, wk, wv, repeats=1)
